# revision 25
# baseline (speedup 1.0000x reference)
"""Trainium2 Bass kernel for nn_Downsample_Spa: sigma-conv + gaussian unfold downsample.

Math (per batch image, one NeuronCore each; batch of 8 -> 8 cores):
  xp = reflect_pad(x)                                  # [64,130,130]
  sigma[o,p] = clamp(BN(conv3x3(xp))[o,p], 1e-4)       # at stride-2 positions p only
  g[o,p]     = exp(-0.5*d2[o]/sigma^2 - ln64) / sigma
  out[c,p]   = sum_o (g[o,p]/S[p]) * xp[c,p+off(o)],  S = sum_o g[o,p]

Design (v3 — PE tile-concurrency + whole-image batching):
 - partitions = (row-half hh, channel c) = 128; host pre-pads (reflect), fp16,
   columns parity-split into 2 planes (w=2j / w=2j+1); the third tap plane
   (w=2j+2 = plane0 shifted one slot) is rebuilt on-chip by idle ACT/DVE
   copies during the input DMA, keeping every tap view step-1 and 4B-aligned.
 - sigma rows laid out r = 32*blk + 9*hh + o: the 4 row-blocks of the image map
   to the 4 PE column-groups, so the conv's 36 matmuls (9 taps x 4 blocks,
   K=128, M=32 padded) run 4-concurrent via tile_position, all into ONE PSUM
   bank [128,512]. The whole g pipeline then runs ONCE on [128,512] tiles.
 - normalization EARLY on the 18-row side: S via ones-matmul (M=8), fast
   reciprocal [8,512] -> bf16, re-broadcast via K=8 bf16 matmul, folded into
   gbn (bf16).
 - unfold: per tap one round of 4 row-group-concurrent one-hot matmuls
   (K=18 at partition base 32b) -> [128,32,64] PSUM, ACT-copied to fp16,
   multiplied with the full-column x view [128,32,64] on DVE (2x mode).
   Pair-tree adds ride in the ACT-paced round slack; tail is 2 adds + split
   fp16 output DMA (host casts to f32).
 - long PE warm-up chain (28 matmuls into a spare PSUM corner) spans the
   input-DMA wait so the conv starts HAM-warm at 2.4 GHz.
"""

import os
import sys

import numpy as np

if "/opt/trn_rl_repo" not in sys.path:
    sys.path.insert(0, "/opt/trn_rl_repo")

K = 3
BN_EPS = 1e-5
SIGMA_MIN = 1e-4
GLN = float(np.log(64.0))
N, C, H, W = 8, 64, 128, 128
HO = WO = 64
RS = 65                  # padded-row slots per partition-half
JW = 66                  # j slots per plane
CONV_ORDER = (0, 1, 3, 4, 6, 7, 2, 5, 8)   # dj==2 taps last (xs2 built on-chip)

_STATE = {}


def _build_consts(conv_w, bn_gamma, bn_beta, bn_mean, bn_var):
    import ml_dtypes
    s = (bn_gamma / np.sqrt(bn_var + BN_EPS)).astype(np.float32)
    wf = conv_w.astype(np.float32) * s[:, None, None, None]           # [9,64,3,3]
    bias = (bn_beta - bn_mean * s).astype(np.float32)
    d2 = np.array([(t // 3 - 1) ** 2 + (t % 3 - 1) ** 2 for t in range(9)], np.float32)

    cst = np.zeros((128, 3), np.float32)
    cst[:, 2] = -GLN
    win = np.zeros((128, 9 * 32), np.float16)
    gh = np.zeros((128, 9 * 128), ml_dtypes.bfloat16)
    s8 = np.zeros((128, 8), ml_dtypes.bfloat16)
    r8 = np.zeros((8, 128), ml_dtypes.bfloat16)

    for t in range(9):
        i, j = t // 3, t % 3
        for h in range(2):
            win[h * 64:(h + 1) * 64, t * 32 + 9 * h: t * 32 + 9 * h + 9] = \
                wf[:, :, i, j].T.astype(np.float16)
    for b in range(4):
        for h in range(2):
            for o in range(9):
                r = 32 * b + 9 * h + o
                cst[r, 0] = -0.5 * d2[o]
                cst[r, 1] = bias[o] - SIGMA_MIN
                gh[r, o * 128 + h * 64: o * 128 + h * 64 + 64] = 1.0
                s8[r, 2 * b + h] = 1.0
                r8[2 * b + h, r] = 1.0
    return cst, win, gh, s8, r8


def _build_bass(for_sim=False):
    import concourse.bass as bass
    import concourse.tile as tile
    from concourse import mybir

    f32 = mybir.dt.float32
    f16 = mybir.dt.float16
    bf16 = mybir.dt.bfloat16
    MULT = mybir.AluOpType.mult
    ADD = mybir.AluOpType.add
    MAX = mybir.AluOpType.max
    AF = mybir.ActivationFunctionType

    if for_sim:
        nc = bass.Bass("TRN2", target_bir_lowering=False, detect_race_conditions=False)
    else:
        from concourse import bacc
        nc = bacc.Bacc()
    xin = nc.dram_tensor("xin", [128, RS, 2, JW], f16, kind="ExternalInput")
    cin = nc.dram_tensor("cin", [128, 3], f32, kind="ExternalInput")
    win = nc.dram_tensor("win", [128, 9 * 32], f16, kind="ExternalInput")
    ghin = nc.dram_tensor("ghin", [128, 9 * 128], bf16, kind="ExternalInput")
    s8in = nc.dram_tensor("s8in", [128, 8], bf16, kind="ExternalInput")
    r8in = nc.dram_tensor("r8in", [8, 128], bf16, kind="ExternalInput")
    out = nc.dram_tensor("out", [128, 32, 64], f16, kind="ExternalOutput")

    with tile.TileContext(nc) as tc:
        from contextlib import ExitStack
        with ExitStack() as ctx:
            big = ctx.enter_context(tc.tile_pool(name="big", bufs=1))
            gp = ctx.enter_context(tc.tile_pool(name="gp", bufs=1))
            gcp = ctx.enter_context(tc.tile_pool(name="gcp", bufs=4))
            yp = ctx.enter_context(tc.tile_pool(name="yp", bufs=1))
            ps = ctx.enter_context(tc.tile_pool(name="ps", bufs=2, space="PSUM"))

            ws = big.tile([128, 9 * 32], f16)
            nc.sync.dma_start(out=ws[:], in_=win[:])
            xs = big.tile([128, RS, 2, JW], f16)
            # two row-contiguous halves: conv on blocks {0,1} starts while
            # blocks {2,3} still stream
            nc.sync.dma_start(out=xs[:, 0:33], in_=xin[:, 0:33])
            nc.sync.dma_start(out=xs[:, 33:65], in_=xin[:, 33:65])
            cs = big.tile([128, 3], f32)
            nc.scalar.dma_start(out=cs[:], in_=cin[:])
            gs = big.tile([128, 9 * 128], bf16)
            nc.scalar.dma_start(out=gs[:], in_=ghin[:])
            s8t = big.tile([128, 8], bf16)
            nc.scalar.dma_start(out=s8t[:], in_=s8in[:])
            r8t = big.tile([8, 128], bf16)
            nc.scalar.dma_start(out=r8t[:], in_=r8in[:])

            # plane2 (w=2j+2) = plane0 shifted one j-slot, rebuilt by the
            # otherwise-idle ACT/DVE engines during the DMA wait
            xs2 = big.tile([128, RS, JW], f16)
            nc.scalar.activation(out=xs2[:, 0:33, 0:65], in_=xs[:, 0:33, 0, 1:66],
                                 func=AF.Copy)
            nc.vector.tensor_copy(out=xs2[:, 33:65, 0:65], in_=xs[:, 33:65, 0, 1:66])

            def xtap(t, r0, nstep, c0=0, c1=64):
                i, dj = t // 3, t % 3
                if dj < 2:
                    return xs[:, r0 + i: r0 + i + 2 * nstep - 1: 2, dj, c0:c1]
                return xs2[:, r0 + i: r0 + i + 2 * nstep - 1: 2, c0:c1]

            # ---- PSUM rotation through two 4-bank slots (tag "ps") ----
            sig = ps.tile([128, 512], f32, tag="ps")        # slot A
            ssb = ps.tile([128, 512], f32, tag="ps")        # slot B

            # PE warm-up chain spanning the input-DMA wait (spare PSUM rows)
            for _ in range(14):
                nc.tensor.matmul(ssb[32:64, 0:256], ws[:, 0:32], ws[:, 0:256],
                                 start=True, stop=True)

            # ---- conv: 9 taps x 4 blocks, col-group concurrent; blocks
            # {0,1} run as soon as the first DMA half lands ----
            for pas in ((0, 1), (2, 3)):
                for k, t in enumerate(CONV_ORDER):
                    for b in pas:
                        nc.tensor.matmul(
                            sig[32 * b:32 * b + 32, :],
                            ws[:, 32 * t:32 * t + 32],
                            xtap(t, 16 * b, 8),
                            start=(k == 0), stop=(k == 8),
                            tile_position=(0, 32 * b),
                        )

            # ---- g pipeline on [128, 512], FD-split in 2 halves so the
            # serial DVE<->ACT chain pipelines ----
            sc = gp.tile([128, 512], f32)
            inv = gp.tile([128, 512], f32)
            qt = gp.tile([128, 512], f32)
            et = gp.tile([128, 512], f32)
            gb = gp.tile([128, 512], bf16)
            for h in (slice(0, 256), slice(256, 512)):
                nc.vector.tensor_scalar(out=sc[:, h], in0=sig[:, h],
                                        scalar1=cs[:, 1:2], scalar2=float(SIGMA_MIN),
                                        op0=ADD, op1=MAX)
                nc.vector.reciprocal_approx_fast(out=inv[:, h], in_=sc[:, h])
                nc.scalar.activation(out=qt[:, h], in_=inv[:, h], func=AF.Square)
                nc.scalar.activation(out=et[:, h], in_=qt[:, h], func=AF.Exp,
                                     scale=cs[:, 0:1], bias=cs[:, 2:3])
                nc.vector.tensor_tensor(out=gb[:, h], in0=et[:, h], in1=inv[:, h],
                                        op=MULT)

            # ---- early normalization, per FD-half so norm-h0 pipelines
            # under the h1 g-chain: S, 1/S (bf16), re-broadcast, fold ----
            rst = gp.tile([8, 512], f32)
            rsb = gp.tile([8, 512], bf16)
            srep = ps.tile([128, 512], f32, tag="ps")       # slot A
            gbn = gp.tile([128, 512], bf16)
            for h in (slice(0, 256), slice(256, 512)):
                nc.tensor.matmul(ssb[0:8, h], s8t[:, 0:8], gb[:, h],
                                 start=True, stop=True)
                nc.vector.reciprocal_approx_fast(out=rst[:, h], in_=ssb[0:8, h])
                nc.vector.tensor_copy(out=rsb[:, h], in_=rst[:, h])
                nc.tensor.matmul(srep[:, h], r8t[:], rsb[:, h],
                                 start=True, stop=True)
                nc.vector.tensor_tensor(out=gbn[:, h], in0=gb[:, h],
                                        in1=srep[:, h], op=MULT)

            # ---- unfold: 9 rounds of 4 row-group-concurrent broadcasts ----
            # half-size (16-row) pair-tree adds ride the DVE slack of the
            # ACT-copy-paced rounds; ta slots: 0=a01 1=a23 2=a45 3=a67
            # 4=q0(a01+a23) 5=q1(a45+a67) 6=s(q0+q1); t0 = s + p8
            yt = yp.tile([128, 9, 32, 64], f16)
            ta = yp.tile([128, 7, 32, 64], f16)
            t0 = yp.tile([128, 32, 64], f16)
            HALVES = (slice(0, 16), slice(16, 32))
            PAIRS = {1: (0, yt[:, 0], yt[:, 1]), 3: (1, yt[:, 2], yt[:, 3]),
                     4: (4, ta[:, 0], ta[:, 1]), 5: (2, yt[:, 4], yt[:, 5]),
                     7: (3, yt[:, 6], yt[:, 7]), 8: (5, ta[:, 2], ta[:, 3])}

            for r in range(9):
                gcps = ps.tile([128, 32, 64], f32, tag="ps")  # slots B,A,B,...
                for b in range(4):
                    nc.tensor.matmul(
                        gcps[:, 8 * b:8 * b + 8, :],
                        gs[32 * b:32 * b + 18, 128 * r:128 * r + 128],
                        gbn[32 * b:32 * b + 18, :],
                        start=True, stop=True,
                        tile_position=(32 * b, 0),
                    )
                gcsb = gcp.tile([128, 32, 64], f16, tag="gc")
                nc.scalar.activation(out=gcsb[:], in_=gcps[:], func=AF.Copy)
                nc.vector.tensor_tensor(out=yt[:, r], in0=xtap(r, 0, 32),
                                        in1=gcsb[:], op=MULT)
                if r in PAIRS:
                    d, i0, i1 = PAIRS[r]
                    for h in HALVES:
                        nc.vector.tensor_tensor(out=ta[:, d, h], in0=i0[:, h],
                                                in1=i1[:, h], op=ADD)

            # ---- tail: s = q0+q1, t0 = s+p8 per 16-row half, DMA ASAP ----
            for h in HALVES:
                nc.vector.tensor_tensor(out=ta[:, 6, h], in0=ta[:, 4, h],
                                        in1=ta[:, 5, h], op=ADD)
                nc.vector.tensor_tensor(out=t0[:, h], in0=ta[:, 6, h],
                                        in1=yt[:, 8, h], op=ADD)
                eng = nc.sync if h.start == 0 else nc.scalar
                eng.dma_start(out=out[:, h], in_=t0[:, h])

    if not for_sim and not nc.is_finalized():
        nc.finalize()
    return nc


def _prep_inputs(x, conv_w, bn_gamma, bn_beta, bn_mean, bn_var):
    cst, win, gh, s8, r8 = _build_consts(conv_w, bn_gamma, bn_beta, bn_mean, bn_var)
    xp = np.pad(np.asarray(x, np.float32), ((0, 0), (0, 0), (1, 1), (1, 1)),
                mode="reflect").astype(np.float16)                    # [8,64,130,130]
    in_maps = []
    for n in range(N):
        xc = np.concatenate([xp[n, :, 0:RS, :], xp[n, :, 64:64 + RS, :]], axis=0)
        xpl = np.zeros((128, RS, 2, JW), np.float16)
        xpl[:, :, 0, 0:65] = xc[:, :, 0:130:2]
        xpl[:, :, 1, 0:65] = xc[:, :, 1:130:2]
        in_maps.append({"xin": xpl, "cin": cst, "win": win, "ghin": gh,
                        "s8in": s8, "r8in": r8})
    return in_maps


def _gather(results):
    out = np.empty((N, C, HO, WO), np.float32)
    for n in range(N):
        d = results[n]["out"].astype(np.float32)
        out[n, :, 0:32, :] = d[0:64]
        out[n, :, 32:, :] = d[64:128]
    return out


def _enable_axon_trace():
    """Register the NTFF profile hook that this image's antenv lacks."""
    if _STATE.get("trace_hooked"):
        return
    import types
    import antenv
    from concourse import bass_utils
    mod = types.ModuleType("antenv.axon_hooks")
    mod._hook = None
    mod.set_axon_ntff_profile_hook = lambda h: setattr(mod, "_hook", h)
    mod.get_axon_ntff_profile_hook = lambda: mod._hook
    sys.modules["antenv.axon_hooks"] = mod
    antenv.axon_hooks = mod
    from trn_agent_boot.trn_boot import _ntff_profile_via_ctypes
    mod._hook = _ntff_profile_via_ctypes("/opt/axon/libaxon_pjrt.so")
    bass_utils.upload_artifacts = lambda tmpdir: tmpdir
    _STATE["trace_hooked"] = True


def run(x, conv_w, bn_gamma, bn_beta, bn_mean, bn_var, trace=False):
    from concourse.bass_utils import run_bass_kernel_spmd
    if trace:
        _enable_axon_trace()
    if "nc" not in _STATE:
        _STATE["nc"] = _build_bass()
    in_maps = _prep_inputs(x, conv_w, bn_gamma, bn_beta, bn_mean, bn_var)
    res = run_bass_kernel_spmd(_STATE["nc"], in_maps, list(range(N)), trace=trace)
    _STATE["last"] = res
    return _gather(res.results)


def kernel(x, conv_w, bn_gamma, bn_beta, bn_mean, bn_var):
    return run(x, conv_w, bn_gamma, bn_beta, bn_mean, bn_var,
               trace=bool(int(os.environ.get("KERNEL_TRACE", "0"))))
